# revision 17
# baseline (speedup 1.0000x reference)
"""AlignBlock kernel for 8 TRN2 NeuronCores.

Reference computation (B=2, C=2, T=500, F=129, H=16, D=100):
  Q = conv1x1(x_mic; w_mic, b_mic)        (B,H,T,F)
  K = conv1x1(x_ref; w_ref, b_ref)        (B,H,T,F)
  V[b,h,t,d]  = sum_f Q[b,h,t,f] * Kpad[b,h,t-99+d,f]       (delay window)
  V2 = conv2d(V, w_conv (1,H,5,3), causal-T pad (4,0), d pad (1,1)) + b_conv
  A  = softmax_d(V2[:,0])                 (B,T,D)
  y[b,c,t,f] = sum_d x_refpad[b,c,t-99+d,f] * A[b,t,d]

Key algebraic restructuring (all exact):
  - The H dimension is folded on the host: with augmented channels
    xm~ = [xm0, xm1, u], xr~ = [xr0, xr1, v] (u/v = validity masks emulating
    the reference's zero padding of Q rows / K columns), the conv input
    planes are sum_h w_conv[h]*V[h] = sum_{q=(cm,cr)} Wc[q] * XC[q] where
    XC[q][t,t'] = sum_f xm~[cm,t,f] xr~[cr,t',f]  (9 raw correlation planes)
    and Wc[q,i,j] = sum_h w_conv[h,i,j] wm~[h,cm] wr~[h,cr].
  - The causal 5-tap T conv becomes banded-matrix matmuls (contraction over
    conv input rows); the 3 d-taps are free-dim shifted column reads.
  - softmax(V2 + b_conv) == softmax(V2): b_conv is dropped.
  - y is a matmul contracting t' with the banded attention matrix A_band.

Sharding: sequence-parallel over T, 63 output frames/core (T padded 500->504),
each core loads its input slice with halos host-side; no collectives.
"""

import os
import sys

import numpy as np

sys.path.insert(0, "/opt/trn_rl_repo")

# ---- problem constants (hardcoded per the staged problem) ----
B, C, T, F = 2, 2, 500, 129
H, D = 16, 100
NCORES = 8
TL = 63               # output frames per core
TP = NCORES * TL      # padded T = 504
R = TL + 4            # conv input rows per core (67)
TH = TL + D + 3       # x_ref halo columns per core (166)
NQ = 9                # augmented channel pairs
DW = D + 2            # padded delay width incl. zero edge cols (102)

_CACHE = {}


def _np_reference(x_mic, x_ref, w_mic, b_mic, w_ref, b_ref, w_conv, b_conv, delay):
    """Pure-numpy fallback, exact mirror of the jax reference."""
    Bn, Cn, Tn, Fn = x_mic.shape
    Dn = int(delay)
    Q = np.einsum("bctf,hc->bhtf", x_mic, w_mic) + b_mic[None, :, None, None]
    K = np.einsum("bctf,hc->bhtf", x_ref, w_ref) + b_ref[None, :, None, None]
    idx = np.arange(Tn)[:, None] + np.arange(Dn)[None, :]
    Kp = np.pad(K, ((0, 0), (0, 0), (Dn - 1, 0), (0, 0)))
    Ku = Kp[:, :, idx, :]
    V = np.einsum("bhtf,bhtdf->bhtd", Q, Ku)
    Hh = w_conv.shape[1]
    Vp = np.pad(V, ((0, 0), (0, 0), (4, 0), (1, 1)))
    out = np.zeros((Bn, Tn, Dn), np.float32)
    for i in range(5):
        for j in range(3):
            out += np.einsum(
                "bhtd,h->btd", Vp[:, :, i : i + Tn, j : j + Dn], w_conv[0, :, i, j]
            )
    out += b_conv[0]
    m = out.max(-1, keepdims=True)
    e = np.exp(out - m)
    A = e / e.sum(-1, keepdims=True)
    Rp = np.pad(x_ref, ((0, 0), (0, 0), (Dn - 1, 0), (0, 0)))
    Ru = Rp[:, :, idx, :]
    return np.einsum("bctdf,btd->bctf", Ru, A).astype(np.float32)


def _build_graph():
    """Build + compile the single-core SPMD Bass graph (identical on all cores)."""
    from concourse import bacc, mybir, tile

    dt = mybir.dt
    f32 = dt.float32
    bf16 = dt.bfloat16

    nc = bacc.Bacc(
        "TRN2", target_bir_lowering=False, debug=False, num_devices=NCORES
    )

    # external I/O (per-core shards, host-prepared layouts)
    xmt = nc.dram_tensor("xmt", [F, B, 3, R], bf16, kind="ExternalInput")
    xrt = nc.dram_tensor("xrt", [F, B, 3, TH], bf16, kind="ExternalInput")
    xrn = nc.dram_tensor("xrn", [TH, B, C, F], bf16, kind="ExternalInput")
    bcv = nc.dram_tensor("bcv", [R, NQ, 3, TL], bf16, kind="ExternalInput")
    out = nc.dram_tensor("out", [B, C, TL, F], f32, kind="ExternalOutput")

    # DRAM scratch for the band->rect diagonal regather
    sxc = nc.dram_tensor("sxc", [B * NQ * R * TH + 64], bf16)
    # row-banded attention scratch, one per batch: sg[tau*256 + t'] band layout
    sg = [nc.dram_tensor(f"sg{b}", [64 * 256], bf16) for b in range(B)]

    VecI64Pair = None

    def strided_ap(handle, offset_el, dims):
        """AP on a flat DRAM tensor with explicit [stride, size] dims."""
        nonlocal VecI64Pair
        a = handle.ap().copy()
        if VecI64Pair is None:
            VecI64Pair = type(a.ap)
        a.ap = VecI64Pair([list(d) for d in dims])
        a.offset = offset_el
        return a

    with tile.TileContext(nc) as tc:
        with (
            tc.tile_pool(name="w", bufs=1) as wp,
            tc.tile_pool(name="xcps", bufs=4, space="PSUM") as xcp,
            tc.tile_pool(name="cvps", bufs=1, space="PSUM") as cvp,
            tc.tile_pool(name="yps", bufs=2, space="PSUM") as yp,
            tc.tile_pool(name="st", bufs=3) as sp,
            tc.tile_pool(name="sm", bufs=2) as smp,
        ):
            # ---- persistent input tiles ----
            xmt0 = wp.tile([128, B, 3, R], bf16, tag="xmt0")
            xmt1 = wp.tile([1, B, 3, R], bf16, tag="xmt1")
            xrt0 = wp.tile([128, B, 3, TH], bf16, tag="xrt0")
            xrt1 = wp.tile([1, B, 3, TH], bf16, tag="xrt1")
            xrn0 = wp.tile([128, B, C, F], bf16, tag="xrn0")
            xrn1 = wp.tile([TH - 128, B, C, F], bf16, tag="xrn1")
            bcw = wp.tile([R, NQ, 3, TL], bf16, tag="bcw")

            # spread the input loads over both HWDGE trigger engines
            nc.sync.dma_start(out=xmt0[:], in_=xmt[0:128])
            nc.sync.dma_start(out=xmt1[:], in_=xmt[128:129])
            nc.sync.dma_start(out=xrt0[:], in_=xrt[0:128])
            nc.sync.dma_start(out=xrt1[:], in_=xrt[128:129])
            nc.scalar.dma_start(out=xrn0[:], in_=xrn[0:128])
            nc.scalar.dma_start(out=xrn1[:], in_=xrn[128:TH])
            nc.scalar.dma_start(out=bcw[:], in_=bcv[:])

            # zero the row-banded attention scratch (band writes fill the rest)
            zsrc = wp.tile([64, B, 256], bf16, tag="zsrc")
            nc.gpsimd.memset(zsrc[:], 0.0)
            for b in range(B):
                nc.gpsimd.dma_start(
                    out=strided_ap(sg[b], 0, [[256, 64], [1, 256]]),
                    in_=zsrc[:, b, :],
                )

            # single rect correlation-plane tile (edge cols stay zero)
            xcd = wp.tile([R, NQ, B, DW], bf16, tag="xcd")
            nc.gpsimd.memset(xcd[:], 0.0)

            # ---- stage 1: 9 correlation planes per batch (band coords) ----
            for b in range(B):
                xsb = sp.tile([R, NQ, TH], bf16, tag="xsb")
                for q in range(NQ):
                    qm, qr = divmod(q, 3)
                    pxc = xcp.tile([R, TH], f32, tag="pxc")
                    nc.tensor.matmul(
                        out=pxc[:], lhsT=xmt0[:, b, qm, :], rhs=xrt0[:, b, qr, :],
                        start=True, stop=False,
                    )
                    nc.tensor.matmul(
                        out=pxc[:], lhsT=xmt1[:, b, qm, :], rhs=xrt1[:, b, qr, :],
                        start=False, stop=True,
                    )
                    nc.vector.tensor_copy(out=xsb[:, q, :], in_=pxc[:])
                off = b * NQ * R * TH
                nc.sync.dma_start(
                    out=strided_ap(
                        sxc, off, [[TH, R], [R * TH, NQ], [1, TH]]
                    ),
                    in_=xsb[:],
                )
                # diagonal regather: XCd[r, q, b, d] = XC[q][r, r + d]
                nc.scalar.dma_start(
                    out=xcd[:, :, b, 1 : 1 + D],
                    in_=strided_ap(
                        sxc, off, [[TH + 1, R], [R * TH, NQ], [1, D]]
                    ),
                )

            # ---- stage 2: folded conv as banded matmuls ----
            v2 = cvp.tile([TL, B, D], f32, tag="v2")
            n_mm = NQ * 3
            k = 0
            for q in range(NQ):
                for j in range(3):
                    nc.tensor.matmul(
                        out=v2[:],
                        lhsT=bcw[:, q, j, :],
                        rhs=xcd[:, q, :, j : j + D],
                        start=(k == 0), stop=(k == n_mm - 1),
                    )
                    k += 1

            # ---- stage 3: softmax over delay, per batch ----
            abt0, abt1 = [], []
            for b in range(B):
                mx = smp.tile([TL, 1], f32, tag="mx")
                nc.vector.tensor_reduce(
                    out=mx[:], in_=v2[:, b, :],
                    axis=mybir.AxisListType.X, op=mybir.AluOpType.max,
                )
                nmx = smp.tile([TL, 1], f32, tag="nmx")
                nc.vector.tensor_scalar_mul(nmx[:], mx[:], -1.0)
                ex = smp.tile([TL, D], f32, tag="ex")
                ssum = smp.tile([TL, 1], f32, tag="ssum")
                nc.scalar.activation(
                    out=ex[:], in_=v2[:, b, :],
                    func=mybir.ActivationFunctionType.Exp,
                    bias=nmx[:], scale=1.0, accum_out=ssum[:],
                )
                rin = smp.tile([TL, 1], f32, tag="rin")
                nc.vector.reciprocal(rin[:], ssum[:])
                att = smp.tile([TL, D], bf16, tag="att")
                nc.vector.tensor_scalar_mul(att[:], ex[:], rin[:])

                # A -> row-banded DRAM scratch -> xbar transpose to [t', tau].
                # sg[tau*256 + tau + 4 + d] = A[tau, d]; rest stays zero.
                eng_w = nc.sync if b == 0 else nc.scalar
                eng_w.dma_start(
                    out=strided_ap(sg[b], 4, [[257, TL], [1, D]]), in_=att[:]
                )
                a0 = wp.tile([128, 64], bf16, tag=f"a0_{b}")
                nc.sync.dma_start_transpose(
                    out=a0[:], in_=strided_ap(sg[b], 0, [[256, 64], [1, 128]])
                )
                a1 = wp.tile([128, 64], bf16, tag=f"a1_{b}")
                nc.scalar.dma_start_transpose(
                    out=a1[:], in_=strided_ap(sg[b], 128, [[256, 64], [1, 128]])
                )
                abt0.append(a0)
                abt1.append(a1)

            # ---- stage 4: delay-weighted sum as banded matmul ----
            yout = wp.tile([TL, B, C, F], f32, tag="yout")
            for b in range(B):
                for c in range(C):
                    py = yp.tile([TL, F], f32, tag="py")
                    nc.tensor.matmul(
                        out=py[:], lhsT=abt0[b][:, 0:TL], rhs=xrn0[:, b, c, :],
                        start=True, stop=False,
                    )
                    nc.tensor.matmul(
                        out=py[:], lhsT=abt1[b][0 : TH - 128, 0:TL],
                        rhs=xrn1[:, b, c, :],
                        start=False, stop=True,
                    )
                    nc.vector.tensor_copy(out=yout[:, b, c, :], in_=py[:])

            nc.gpsimd.dma_start(
                out=out.ap().transpose([2, 0, 1, 3]), in_=yout[:]
            )

    nc.compile()
    return nc


def _prepare_inputs(x_mic, x_ref, w_mic, b_mic, w_ref, b_ref, w_conv):
    """Host-side sharding + weight folding. Returns in_maps (one dict/core)."""
    from ml_dtypes import bfloat16

    # padded arrays: xm rows [t0-4, t0+63), xr cols [t0-103, t0+63)
    xm_pad = np.zeros((B, C, 4 + TP, F), np.float32)
    xm_pad[:, :, 4 : 4 + T] = x_mic
    xr_pad = np.zeros((B, C, D + 3 + TP, F), np.float32)
    xr_pad[:, :, D + 3 : D + 3 + T] = x_ref

    # folded conv weights: Wc[cm, cr, i, j] = sum_h w_conv * wm~ * wr~
    wt = np.asarray(w_conv, np.float64)[0]          # (H, 5, 3)
    wtm = np.concatenate([w_mic, b_mic[:, None]], 1).astype(np.float64)  # (H,3)
    wtr = np.concatenate([w_ref, b_ref[:, None]], 1).astype(np.float64)  # (H,3)
    Wc = np.einsum("hij,hm,hr->mrij", wt, wtm, wtr)  # (3,3,5,3)

    # banded conv matrices bcv[r, q, j, tau] = Wc[q, r-tau, j]
    bcv = np.zeros((R, 3, 3, 3, TL), np.float32)
    for i in range(5):
        for j in range(3):
            bcv[np.arange(TL) + i, :, :, j, np.arange(TL)] = np.float32(
                Wc[:, :, i, j]
            )[None]
    bcv = bcv.reshape(R, NQ, 3, TL).astype(bfloat16)

    in_maps = []
    for i in range(NCORES):
        t0 = i * TL
        xm_s = xm_pad[:, :, t0 : t0 + R]          # (B,C,R,F) rows t0-4..t0+62
        xr_s = xr_pad[:, :, t0 : t0 + TH]         # (B,C,TH,F) cols t0-103..t0+62
        u = (np.arange(R) + t0 - 4 >= 0).astype(np.float32)
        v = (np.arange(TH) + t0 - D - 3 >= 0).astype(np.float32)

        xmt = np.empty((B, 3, R, F), np.float32)
        xmt[:, :C] = xm_s
        xmt[:, C] = u[:, None]
        xmt = np.ascontiguousarray(xmt.transpose(3, 0, 1, 2)).astype(bfloat16)

        xrt = np.empty((B, 3, TH, F), np.float32)
        xrt[:, :C] = xr_s
        xrt[:, C] = v[:, None]
        xrt = np.ascontiguousarray(xrt.transpose(3, 0, 1, 2)).astype(bfloat16)

        xrn = np.ascontiguousarray(xr_s.transpose(2, 0, 1, 3)).astype(bfloat16)

        in_maps.append({"xmt": xmt, "xrt": xrt, "xrn": xrn, "bcv": bcv})
    return in_maps


def kernel(**inputs):
    x_mic = np.asarray(inputs["x_mic"], np.float32)
    x_ref = np.asarray(inputs["x_ref"], np.float32)
    w_mic = np.asarray(inputs["w_mic"], np.float32)
    b_mic = np.asarray(inputs["b_mic"], np.float32)
    w_ref = np.asarray(inputs["w_ref"], np.float32)
    b_ref = np.asarray(inputs["b_ref"], np.float32)
    w_conv = np.asarray(inputs["w_conv"], np.float32)
    b_conv = np.asarray(inputs["b_conv"], np.float32)
    delay = int(np.asarray(inputs["delay"]))

    if (
        x_mic.shape != (B, C, T, F)
        or x_ref.shape != (B, C, T, F)
        or delay != D
        or w_conv.shape != (1, H, 5, 3)
    ):
        return _np_reference(
            x_mic, x_ref, w_mic, b_mic, w_ref, b_ref, w_conv, b_conv, delay
        )

    from concourse.bass_utils import run_bass_kernel_spmd

    if "nc" not in _CACHE:
        _CACHE["nc"] = _build_graph()
    nc = _CACHE["nc"]

    in_maps = _prepare_inputs(x_mic, x_ref, w_mic, b_mic, w_ref, b_ref, w_conv)
    res = run_bass_kernel_spmd(nc, in_maps, core_ids=list(range(NCORES)))

    y = np.zeros((B, C, TP, F), np.float32)
    for i in range(NCORES):
        y[:, :, i * TL : (i + 1) * TL] = res.results[i]["out"]
    return np.ascontiguousarray(y[:, :, :T]).astype(np.float32)


if __name__ == "__main__":
    rng = np.random.default_rng(0)
    ins = {
        "x_mic": rng.standard_normal((B, C, T, F), np.float32),
        "x_ref": rng.standard_normal((B, C, T, F), np.float32),
        "w_mic": rng.standard_normal((H, C), np.float32) * 0.5,
        "b_mic": rng.standard_normal((H,), np.float32) * 0.1,
        "w_ref": rng.standard_normal((H, C), np.float32) * 0.5,
        "b_ref": rng.standard_normal((H,), np.float32) * 0.1,
        "w_conv": rng.standard_normal((1, H, 5, 3), np.float32) * 0.05,
        "b_conv": rng.standard_normal((1,), np.float32) * 0.1,
        "delay": D,
    }
    got = kernel(**ins)
    want = _np_reference(**ins)
    err = np.linalg.norm(got - want) / np.linalg.norm(want)
    print("rel err vs numpy ref:", err)


# revision 18
# speedup vs baseline: 1.2298x; 1.2298x over previous
"""AlignBlock kernel for 8 TRN2 NeuronCores.

Reference computation (B=2, C=2, T=500, F=129, H=16, D=100):
  Q = conv1x1(x_mic; w_mic, b_mic)        (B,H,T,F)
  K = conv1x1(x_ref; w_ref, b_ref)        (B,H,T,F)
  V[b,h,t,d]  = sum_f Q[b,h,t,f] * Kpad[b,h,t-99+d,f]       (delay window)
  V2 = conv2d(V, w_conv (1,H,5,3), causal-T pad (4,0), d pad (1,1)) + b_conv
  A  = softmax_d(V2[:,0])                 (B,T,D)
  y[b,c,t,f] = sum_d x_refpad[b,c,t-99+d,f] * A[b,t,d]

Key algebraic restructuring (all exact):
  - The H dimension is folded on the host: with augmented channels
    xm~ = [xm0, xm1, u], xr~ = [xr0, xr1, v] (u/v = validity masks emulating
    the reference's zero padding of Q rows / K columns), the conv input
    planes are sum_h w_conv[h]*V[h] = sum_{q=(cm,cr)} Wc[q] * XC[q] where
    XC[q][t,t'] = sum_f xm~[cm,t,f] xr~[cr,t',f]  (9 raw correlation planes)
    and Wc[q,i,j] = sum_h w_conv[h,i,j] wm~[h,cm] wr~[h,cr].
  - The causal 5-tap T conv becomes banded-matrix matmuls (contraction over
    conv input rows); the 3 d-taps are free-dim shifted column reads.
  - softmax(V2 + b_conv) == softmax(V2): b_conv is dropped.
  - y is a matmul contracting t' with the banded attention matrix A_band.

Sharding: sequence-parallel over T, 63 output frames/core (T padded 500->504),
each core loads its input slice with halos host-side; no collectives.
"""

import os
import sys

import numpy as np

sys.path.insert(0, "/opt/trn_rl_repo")

# ---- problem constants (hardcoded per the staged problem) ----
B, C, T, F = 2, 2, 500, 129
H, D = 16, 100
NCORES = 8
TL = 63               # output frames per core
TP = NCORES * TL      # padded T = 504
R = TL + 4            # conv input rows per core (67)
TH = TL + D + 3       # x_ref halo columns per core (166)
NQ = 9                # augmented channel pairs
DW = D + 2            # padded delay width incl. zero edge cols (102)

_CACHE = {}


def _np_reference(x_mic, x_ref, w_mic, b_mic, w_ref, b_ref, w_conv, b_conv, delay):
    """Pure-numpy fallback, exact mirror of the jax reference."""
    Bn, Cn, Tn, Fn = x_mic.shape
    Dn = int(delay)
    Q = np.einsum("bctf,hc->bhtf", x_mic, w_mic) + b_mic[None, :, None, None]
    K = np.einsum("bctf,hc->bhtf", x_ref, w_ref) + b_ref[None, :, None, None]
    idx = np.arange(Tn)[:, None] + np.arange(Dn)[None, :]
    Kp = np.pad(K, ((0, 0), (0, 0), (Dn - 1, 0), (0, 0)))
    Ku = Kp[:, :, idx, :]
    V = np.einsum("bhtf,bhtdf->bhtd", Q, Ku)
    Hh = w_conv.shape[1]
    Vp = np.pad(V, ((0, 0), (0, 0), (4, 0), (1, 1)))
    out = np.zeros((Bn, Tn, Dn), np.float32)
    for i in range(5):
        for j in range(3):
            out += np.einsum(
                "bhtd,h->btd", Vp[:, :, i : i + Tn, j : j + Dn], w_conv[0, :, i, j]
            )
    out += b_conv[0]
    m = out.max(-1, keepdims=True)
    e = np.exp(out - m)
    A = e / e.sum(-1, keepdims=True)
    Rp = np.pad(x_ref, ((0, 0), (0, 0), (Dn - 1, 0), (0, 0)))
    Ru = Rp[:, :, idx, :]
    return np.einsum("bctdf,btd->bctf", Ru, A).astype(np.float32)


def _build_graph():
    """Build + compile the single-core SPMD Bass graph (identical on all cores)."""
    from concourse import bacc, mybir, tile

    dt = mybir.dt
    f32 = dt.float32
    bf16 = dt.bfloat16

    nc = bacc.Bacc(
        "TRN2", target_bir_lowering=False, debug=False, num_devices=NCORES
    )

    # external I/O (per-core shards, host-prepared layouts)
    xmt = nc.dram_tensor("xmt", [F, B, 3, R], bf16, kind="ExternalInput")
    xrt = nc.dram_tensor("xrt", [F, B, 3, TH], bf16, kind="ExternalInput")
    xrn = nc.dram_tensor("xrn", [TH, B, C, F], bf16, kind="ExternalInput")
    bcv = nc.dram_tensor("bcv", [R, NQ, 3, TL], bf16, kind="ExternalInput")
    out = nc.dram_tensor("out", [B, C, TL, F], f32, kind="ExternalOutput")

    # DRAM scratch for the band->rect diagonal regather, one per batch.
    # Layout (r, q, c): whole-partition contiguous rows for the band write.
    sxc = [nc.dram_tensor(f"sxc{b}", [R * NQ * TH + 64], bf16) for b in range(B)]
    # row-banded attention scratch, one per batch: sg[tau*256 + t'] band layout
    sg = [nc.dram_tensor(f"sg{b}", [64 * 256], bf16) for b in range(B)]

    VecI64Pair = None

    def strided_ap(handle, offset_el, dims):
        """AP on a flat DRAM tensor with explicit [stride, size] dims."""
        nonlocal VecI64Pair
        a = handle.ap().copy()
        if VecI64Pair is None:
            VecI64Pair = type(a.ap)
        a.ap = VecI64Pair([list(d) for d in dims])
        a.offset = offset_el
        return a

    with tile.TileContext(nc) as tc:
        with (
            tc.tile_pool(name="w", bufs=1) as wp,
            tc.tile_pool(name="xcps", bufs=4, space="PSUM") as xcp,
            tc.tile_pool(name="cvps", bufs=1, space="PSUM") as cvp,
            tc.tile_pool(name="yps", bufs=2, space="PSUM") as yp,
            tc.tile_pool(name="st", bufs=3) as sp,
            tc.tile_pool(name="sm", bufs=2) as smp,
        ):
            # ---- persistent input tiles ----
            xmt0 = wp.tile([128, B, 3, R], bf16, tag="xmt0")
            xmt1 = wp.tile([1, B, 3, R], bf16, tag="xmt1")
            xrt0 = wp.tile([128, B, 3, TH], bf16, tag="xrt0")
            xrt1 = wp.tile([1, B, 3, TH], bf16, tag="xrt1")
            xrn0 = wp.tile([128, B, C, F], bf16, tag="xrn0")
            xrn1 = wp.tile([TH - 128, B, C, F], bf16, tag="xrn1")
            bcw = wp.tile([R, NQ, 3, TL], bf16, tag="bcw")

            # spread the input loads over both HWDGE trigger engines
            nc.sync.dma_start(out=xmt0[:], in_=xmt[0:128])
            nc.sync.dma_start(out=xmt1[:], in_=xmt[128:129])
            nc.sync.dma_start(out=xrt0[:], in_=xrt[0:128])
            nc.sync.dma_start(out=xrt1[:], in_=xrt[128:129])
            nc.scalar.dma_start(out=xrn0[:], in_=xrn[0:128])
            nc.scalar.dma_start(out=xrn1[:], in_=xrn[128:TH])
            nc.scalar.dma_start(out=bcw[:], in_=bcv[:])

            # zero the row-banded attention scratch (band writes fill the rest)
            zsrc = wp.tile([64, B, 256], bf16, tag="zsrc")
            nc.gpsimd.memset(zsrc[:], 0.0)
            for b in range(B):
                nc.gpsimd.dma_start(
                    out=strided_ap(sg[b], 0, [[256, 64], [1, 256]]),
                    in_=zsrc[:, b, :],
                )

            # single rect correlation-plane tile (edge cols stay zero)
            xcd = wp.tile([R, NQ, B, DW], bf16, tag="xcd")
            nc.gpsimd.memset(xcd[:], 0.0)

            # ---- stage 1: 9 correlation planes per batch (band coords) ----
            for b in range(B):
                xsb = sp.tile([R, NQ, TH], bf16, tag="xsb")
                for q in range(NQ):
                    qm, qr = divmod(q, 3)
                    pxc = xcp.tile([R, TH], f32, tag="pxc")
                    nc.tensor.matmul(
                        out=pxc[:], lhsT=xmt0[:, b, qm, :], rhs=xrt0[:, b, qr, :],
                        start=True, stop=False,
                    )
                    nc.tensor.matmul(
                        out=pxc[:], lhsT=xmt1[:, b, qm, :], rhs=xrt1[:, b, qr, :],
                        start=False, stop=True,
                    )
                    nc.vector.tensor_copy(out=xsb[:, q, :], in_=pxc[:])
                nc.sync.dma_start(
                    out=strided_ap(sxc[b], 0, [[NQ * TH, R], [1, NQ * TH]]),
                    in_=xsb[:],
                )
                # diagonal regather: XCd[r, q, b, d] = XC[q][r, r + d]
                nc.scalar.dma_start(
                    out=xcd[:, :, b, 1 : 1 + D],
                    in_=strided_ap(
                        sxc[b], 0, [[NQ * TH + 1, R], [TH, NQ], [1, D]]
                    ),
                )

            # ---- stage 2: folded conv as banded matmuls ----
            v2 = cvp.tile([TL, B, D], f32, tag="v2")
            n_mm = NQ * 3
            k = 0
            for q in range(NQ):
                for j in range(3):
                    nc.tensor.matmul(
                        out=v2[:],
                        lhsT=bcw[:, q, j, :],
                        rhs=xcd[:, q, :, j : j + D],
                        start=(k == 0), stop=(k == n_mm - 1),
                    )
                    k += 1

            # ---- stage 3: softmax over delay, per batch ----
            abt0, abt1 = [], []
            for b in range(B):
                mx = smp.tile([TL, 1], f32, tag="mx")
                nc.vector.tensor_reduce(
                    out=mx[:], in_=v2[:, b, :],
                    axis=mybir.AxisListType.X, op=mybir.AluOpType.max,
                )
                nmx = smp.tile([TL, 1], f32, tag="nmx")
                nc.vector.tensor_scalar_mul(nmx[:], mx[:], -1.0)
                ex = smp.tile([TL, D], f32, tag="ex")
                ssum = smp.tile([TL, 1], f32, tag="ssum")
                nc.scalar.activation(
                    out=ex[:], in_=v2[:, b, :],
                    func=mybir.ActivationFunctionType.Exp,
                    bias=nmx[:], scale=1.0, accum_out=ssum[:],
                )
                rin = smp.tile([TL, 1], f32, tag="rin")
                nc.vector.reciprocal(rin[:], ssum[:])
                att = smp.tile([TL, D], bf16, tag="att")
                nc.vector.tensor_scalar_mul(att[:], ex[:], rin[:])

                # A -> row-banded DRAM scratch -> xbar transpose to [t', tau].
                # sg[tau*256 + tau + 4 + d] = A[tau, d]; rest stays zero.
                eng_w = nc.sync if b == 0 else nc.scalar
                eng_w.dma_start(
                    out=strided_ap(sg[b], 4, [[257, TL], [1, D]]), in_=att[:]
                )
                a0 = wp.tile([128, 64], bf16, tag=f"a0_{b}")
                nc.sync.dma_start_transpose(
                    out=a0[:], in_=strided_ap(sg[b], 0, [[256, 64], [1, 128]])
                )
                a1 = wp.tile([128, 64], bf16, tag=f"a1_{b}")
                nc.scalar.dma_start_transpose(
                    out=a1[:], in_=strided_ap(sg[b], 128, [[256, 64], [1, 128]])
                )
                abt0.append(a0)
                abt1.append(a1)

            # ---- stage 4: delay-weighted sum as banded matmul ----
            yout = wp.tile([TL, B, C, F], f32, tag="yout")
            for b in range(B):
                for c in range(C):
                    py = yp.tile([TL, F], f32, tag="py")
                    nc.tensor.matmul(
                        out=py[:], lhsT=abt0[b][:, 0:TL], rhs=xrn0[:, b, c, :],
                        start=True, stop=False,
                    )
                    nc.tensor.matmul(
                        out=py[:], lhsT=abt1[b][0 : TH - 128, 0:TL],
                        rhs=xrn1[:, b, c, :],
                        start=False, stop=True,
                    )
                    nc.vector.tensor_copy(out=yout[:, b, c, :], in_=py[:])

            nc.gpsimd.dma_start(
                out=out.ap().transpose([2, 0, 1, 3]), in_=yout[:]
            )

    nc.compile()
    return nc


def _prepare_inputs(x_mic, x_ref, w_mic, b_mic, w_ref, b_ref, w_conv):
    """Host-side sharding + weight folding. Returns in_maps (one dict/core)."""
    from ml_dtypes import bfloat16

    # padded arrays: xm rows [t0-4, t0+63), xr cols [t0-103, t0+63)
    xm_pad = np.zeros((B, C, 4 + TP, F), np.float32)
    xm_pad[:, :, 4 : 4 + T] = x_mic
    xr_pad = np.zeros((B, C, D + 3 + TP, F), np.float32)
    xr_pad[:, :, D + 3 : D + 3 + T] = x_ref

    # folded conv weights: Wc[cm, cr, i, j] = sum_h w_conv * wm~ * wr~
    wt = np.asarray(w_conv, np.float64)[0]          # (H, 5, 3)
    wtm = np.concatenate([w_mic, b_mic[:, None]], 1).astype(np.float64)  # (H,3)
    wtr = np.concatenate([w_ref, b_ref[:, None]], 1).astype(np.float64)  # (H,3)
    Wc = np.einsum("hij,hm,hr->mrij", wt, wtm, wtr)  # (3,3,5,3)

    # banded conv matrices bcv[r, q, j, tau] = Wc[q, r-tau, j]
    bcv = np.zeros((R, 3, 3, 3, TL), np.float32)
    for i in range(5):
        for j in range(3):
            bcv[np.arange(TL) + i, :, :, j, np.arange(TL)] = np.float32(
                Wc[:, :, i, j]
            )[None]
    bcv = bcv.reshape(R, NQ, 3, TL).astype(bfloat16)

    in_maps = []
    for i in range(NCORES):
        t0 = i * TL
        xm_s = xm_pad[:, :, t0 : t0 + R]          # (B,C,R,F) rows t0-4..t0+62
        xr_s = xr_pad[:, :, t0 : t0 + TH]         # (B,C,TH,F) cols t0-103..t0+62
        u = (np.arange(R) + t0 - 4 >= 0).astype(np.float32)
        v = (np.arange(TH) + t0 - D - 3 >= 0).astype(np.float32)

        xmt = np.empty((B, 3, R, F), np.float32)
        xmt[:, :C] = xm_s
        xmt[:, C] = u[:, None]
        xmt = np.ascontiguousarray(xmt.transpose(3, 0, 1, 2)).astype(bfloat16)

        xrt = np.empty((B, 3, TH, F), np.float32)
        xrt[:, :C] = xr_s
        xrt[:, C] = v[:, None]
        xrt = np.ascontiguousarray(xrt.transpose(3, 0, 1, 2)).astype(bfloat16)

        xrn = np.ascontiguousarray(xr_s.transpose(2, 0, 1, 3)).astype(bfloat16)

        in_maps.append({"xmt": xmt, "xrt": xrt, "xrn": xrn, "bcv": bcv})
    return in_maps


def kernel(**inputs):
    x_mic = np.asarray(inputs["x_mic"], np.float32)
    x_ref = np.asarray(inputs["x_ref"], np.float32)
    w_mic = np.asarray(inputs["w_mic"], np.float32)
    b_mic = np.asarray(inputs["b_mic"], np.float32)
    w_ref = np.asarray(inputs["w_ref"], np.float32)
    b_ref = np.asarray(inputs["b_ref"], np.float32)
    w_conv = np.asarray(inputs["w_conv"], np.float32)
    b_conv = np.asarray(inputs["b_conv"], np.float32)
    delay = int(np.asarray(inputs["delay"]))

    if (
        x_mic.shape != (B, C, T, F)
        or x_ref.shape != (B, C, T, F)
        or delay != D
        or w_conv.shape != (1, H, 5, 3)
    ):
        return _np_reference(
            x_mic, x_ref, w_mic, b_mic, w_ref, b_ref, w_conv, b_conv, delay
        )

    from concourse.bass_utils import run_bass_kernel_spmd

    if "nc" not in _CACHE:
        _CACHE["nc"] = _build_graph()
    nc = _CACHE["nc"]

    in_maps = _prepare_inputs(x_mic, x_ref, w_mic, b_mic, w_ref, b_ref, w_conv)
    res = run_bass_kernel_spmd(nc, in_maps, core_ids=list(range(NCORES)))

    y = np.zeros((B, C, TP, F), np.float32)
    for i in range(NCORES):
        y[:, :, i * TL : (i + 1) * TL] = res.results[i]["out"]
    return np.ascontiguousarray(y[:, :, :T]).astype(np.float32)


if __name__ == "__main__":
    rng = np.random.default_rng(0)
    ins = {
        "x_mic": rng.standard_normal((B, C, T, F), np.float32),
        "x_ref": rng.standard_normal((B, C, T, F), np.float32),
        "w_mic": rng.standard_normal((H, C), np.float32) * 0.5,
        "b_mic": rng.standard_normal((H,), np.float32) * 0.1,
        "w_ref": rng.standard_normal((H, C), np.float32) * 0.5,
        "b_ref": rng.standard_normal((H,), np.float32) * 0.1,
        "w_conv": rng.standard_normal((1, H, 5, 3), np.float32) * 0.05,
        "b_conv": rng.standard_normal((1,), np.float32) * 0.1,
        "delay": D,
    }
    got = kernel(**ins)
    want = _np_reference(**ins)
    err = np.linalg.norm(got - want) / np.linalg.norm(want)
    print("rel err vs numpy ref:", err)


# revision 19
# speedup vs baseline: 1.2634x; 1.0273x over previous
"""AlignBlock kernel for 8 TRN2 NeuronCores.

Reference computation (B=2, C=2, T=500, F=129, H=16, D=100):
  Q = conv1x1(x_mic; w_mic, b_mic)        (B,H,T,F)
  K = conv1x1(x_ref; w_ref, b_ref)        (B,H,T,F)
  V[b,h,t,d]  = sum_f Q[b,h,t,f] * Kpad[b,h,t-99+d,f]       (delay window)
  V2 = conv2d(V, w_conv (1,H,5,3), causal-T pad (4,0), d pad (1,1)) + b_conv
  A  = softmax_d(V2[:,0])                 (B,T,D)
  y[b,c,t,f] = sum_d x_refpad[b,c,t-99+d,f] * A[b,t,d]

Key algebraic restructuring (all exact):
  - The H dimension is folded on the host: with augmented channels
    xm~ = [xm0, xm1, u], xr~ = [xr0, xr1, v] (u/v = validity masks emulating
    the reference's zero padding of Q rows / K columns), the conv input
    planes are sum_h w_conv[h]*V[h] = sum_{q=(cm,cr)} Wc[q] * XC[q] where
    XC[q][t,t'] = sum_f xm~[cm,t,f] xr~[cr,t',f]  (9 raw correlation planes)
    and Wc[q,i,j] = sum_h w_conv[h,i,j] wm~[h,cm] wr~[h,cr].
  - The causal 5-tap T conv becomes banded-matrix matmuls (contraction over
    conv input rows); the 3 d-taps are free-dim shifted column reads.
  - softmax(V2 + b_conv) == softmax(V2): b_conv is dropped.
  - y is a matmul contracting t' with the banded attention matrix A_band.

Sharding: sequence-parallel over T, 63 output frames/core (T padded 500->504),
each core loads its input slice with halos host-side; no collectives.
"""

import os
import sys

import numpy as np

sys.path.insert(0, "/opt/trn_rl_repo")

# ---- problem constants (hardcoded per the staged problem) ----
B, C, T, F = 2, 2, 500, 129
H, D = 16, 100
NCORES = 8
TL = 63               # output frames per core
TP = NCORES * TL      # padded T = 504
R = TL + 4            # conv input rows per core (67)
TH = TL + D + 3       # x_ref halo columns per core (166)
NQ = 9                # augmented channel pairs
DW = D + 2            # padded delay width incl. zero edge cols (102)

_CACHE = {}


def _np_reference(x_mic, x_ref, w_mic, b_mic, w_ref, b_ref, w_conv, b_conv, delay):
    """Pure-numpy fallback, exact mirror of the jax reference."""
    Bn, Cn, Tn, Fn = x_mic.shape
    Dn = int(delay)
    Q = np.einsum("bctf,hc->bhtf", x_mic, w_mic) + b_mic[None, :, None, None]
    K = np.einsum("bctf,hc->bhtf", x_ref, w_ref) + b_ref[None, :, None, None]
    idx = np.arange(Tn)[:, None] + np.arange(Dn)[None, :]
    Kp = np.pad(K, ((0, 0), (0, 0), (Dn - 1, 0), (0, 0)))
    Ku = Kp[:, :, idx, :]
    V = np.einsum("bhtf,bhtdf->bhtd", Q, Ku)
    Hh = w_conv.shape[1]
    Vp = np.pad(V, ((0, 0), (0, 0), (4, 0), (1, 1)))
    out = np.zeros((Bn, Tn, Dn), np.float32)
    for i in range(5):
        for j in range(3):
            out += np.einsum(
                "bhtd,h->btd", Vp[:, :, i : i + Tn, j : j + Dn], w_conv[0, :, i, j]
            )
    out += b_conv[0]
    m = out.max(-1, keepdims=True)
    e = np.exp(out - m)
    A = e / e.sum(-1, keepdims=True)
    Rp = np.pad(x_ref, ((0, 0), (0, 0), (Dn - 1, 0), (0, 0)))
    Ru = Rp[:, :, idx, :]
    return np.einsum("bctdf,btd->bctf", Ru, A).astype(np.float32)


def _build_graph():
    """Build + compile the single-core SPMD Bass graph (identical on all cores)."""
    from concourse import bacc, mybir, tile

    dt = mybir.dt
    f32 = dt.float32
    bf16 = dt.bfloat16

    nc = bacc.Bacc(
        "TRN2", target_bir_lowering=False, debug=False, num_devices=NCORES
    )

    # external I/O (per-core shards, host-prepared layouts)
    xmt = nc.dram_tensor("xmt", [F, B, 3, R], bf16, kind="ExternalInput")
    xrt = nc.dram_tensor("xrt", [F, B, 3, TH], bf16, kind="ExternalInput")
    xrn = nc.dram_tensor("xrn", [TH, B, C, F], bf16, kind="ExternalInput")
    bcv = nc.dram_tensor("bcv", [R, NQ, 3, TL], bf16, kind="ExternalInput")
    out = nc.dram_tensor("out", [B, C, TL, F], f32, kind="ExternalOutput")

    # DRAM scratch for the band->rect diagonal regather, one per batch.
    # Layout (r, q, c): whole-partition contiguous rows for the band write.
    sxc = [nc.dram_tensor(f"sxc{b}", [R * NQ * TH + 64], bf16) for b in range(B)]
    # row-banded attention scratch, one per batch: sg[tau*256 + t'] band layout
    sg = [nc.dram_tensor(f"sg{b}", [64 * 256], bf16) for b in range(B)]

    VecI64Pair = None

    def strided_ap(handle, offset_el, dims):
        """AP on a flat DRAM tensor with explicit [stride, size] dims."""
        nonlocal VecI64Pair
        a = handle.ap().copy()
        if VecI64Pair is None:
            VecI64Pair = type(a.ap)
        a.ap = VecI64Pair([list(d) for d in dims])
        a.offset = offset_el
        return a

    with tile.TileContext(nc) as tc:
        with (
            tc.tile_pool(name="w", bufs=1) as wp,
            tc.tile_pool(name="xcps", bufs=4, space="PSUM") as xcp,
            tc.tile_pool(name="cvps", bufs=1, space="PSUM") as cvp,
            tc.tile_pool(name="yps", bufs=2, space="PSUM") as yp,
            tc.tile_pool(name="st", bufs=3) as sp,
            tc.tile_pool(name="sm", bufs=2) as smp,
        ):
            # ---- persistent input tiles ----
            xmt0 = wp.tile([128, B, 3, R], bf16, tag="xmt0")
            xmt1 = wp.tile([1, B, 3, R], bf16, tag="xmt1")
            xrt0 = wp.tile([128, B, 3, TH], bf16, tag="xrt0")
            xrt1 = wp.tile([1, B, 3, TH], bf16, tag="xrt1")
            xrn0 = wp.tile([128, B, C, F], bf16, tag="xrn0")
            xrn1 = wp.tile([TH - 128, B, C, F], bf16, tag="xrn1")
            bcw = wp.tile([R, NQ, 3, TL], bf16, tag="bcw")

            # spread the input loads over both HWDGE trigger engines
            nc.sync.dma_start(out=xmt0[:], in_=xmt[0:128])
            nc.sync.dma_start(out=xmt1[:], in_=xmt[128:129])
            nc.sync.dma_start(out=xrt0[:], in_=xrt[0:128])
            nc.sync.dma_start(out=xrt1[:], in_=xrt[128:129])
            nc.scalar.dma_start(out=xrn0[:], in_=xrn[0:128])
            nc.scalar.dma_start(out=xrn1[:], in_=xrn[128:TH])
            nc.scalar.dma_start(out=bcw[:], in_=bcv[:])

            # zero the row-banded attention scratch (band writes fill the rest)
            zsrc = wp.tile([64, B, 256], bf16, tag="zsrc")
            nc.gpsimd.memset(zsrc[:], 0.0)
            for b in range(B):
                nc.gpsimd.dma_start(
                    out=strided_ap(sg[b], 0, [[256, 64], [1, 256]]),
                    in_=zsrc[:, b, :],
                )

            # single rect correlation-plane tile (edge cols stay zero)
            xcd = wp.tile([R, NQ, B, DW], bf16, tag="xcd")
            nc.gpsimd.memset(xcd[:], 0.0)

            # ---- stage 1: 9 correlation planes per batch (band coords) ----
            for b in range(B):
                xsb = sp.tile([R, NQ, TH], bf16, tag="xsb")
                for q in range(NQ):
                    qm, qr = divmod(q, 3)
                    pxc = xcp.tile([R, TH], f32, tag="pxc")
                    nc.tensor.matmul(
                        out=pxc[:], lhsT=xmt0[:, b, qm, :], rhs=xrt0[:, b, qr, :],
                        start=True, stop=False,
                    )
                    nc.tensor.matmul(
                        out=pxc[:], lhsT=xmt1[:, b, qm, :], rhs=xrt1[:, b, qr, :],
                        start=False, stop=True,
                    )
                    nc.vector.tensor_copy(out=xsb[:, q, :], in_=pxc[:])
                nc.sync.dma_start(
                    out=strided_ap(sxc[b], 0, [[NQ * TH, R], [1, NQ * TH]]),
                    in_=xsb[:],
                )
                # diagonal regather: XCd[r, q, b, d] = XC[q][r, r + d],
                # split into row chunks across the three DMA trigger engines
                for eng, r0, r1 in (
                    (nc.scalar, 0, 23), (nc.sync, 23, 45), (nc.gpsimd, 45, R),
                ):
                    eng.dma_start(
                        out=xcd[r0:r1, :, b, 1 : 1 + D],
                        in_=strided_ap(
                            sxc[b],
                            r0 * (NQ * TH + 1),
                            [[NQ * TH + 1, r1 - r0], [TH, NQ], [1, D]],
                        ),
                    )

            # ---- stage 2: folded conv as banded matmuls ----
            v2 = cvp.tile([TL, B, D], f32, tag="v2")
            n_mm = NQ * 3
            k = 0
            for q in range(NQ):
                for j in range(3):
                    nc.tensor.matmul(
                        out=v2[:],
                        lhsT=bcw[:, q, j, :],
                        rhs=xcd[:, q, :, j : j + D],
                        start=(k == 0), stop=(k == n_mm - 1),
                    )
                    k += 1

            # ---- stage 3: softmax over delay, per batch ----
            abt0, abt1 = [], []
            for b in range(B):
                mx = smp.tile([TL, 1], f32, tag="mx")
                nc.vector.tensor_reduce(
                    out=mx[:], in_=v2[:, b, :],
                    axis=mybir.AxisListType.X, op=mybir.AluOpType.max,
                )
                nmx = smp.tile([TL, 1], f32, tag="nmx")
                nc.vector.tensor_scalar_mul(nmx[:], mx[:], -1.0)
                ex = smp.tile([TL, D], f32, tag="ex")
                ssum = smp.tile([TL, 1], f32, tag="ssum")
                nc.scalar.activation(
                    out=ex[:], in_=v2[:, b, :],
                    func=mybir.ActivationFunctionType.Exp,
                    bias=nmx[:], scale=1.0, accum_out=ssum[:],
                )
                rin = smp.tile([TL, 1], f32, tag="rin")
                nc.vector.reciprocal(rin[:], ssum[:])
                att = smp.tile([TL, D], bf16, tag="att")
                nc.vector.tensor_scalar_mul(att[:], ex[:], rin[:])

                # A -> row-banded DRAM scratch -> xbar transpose to [t', tau].
                # sg[tau*256 + tau + 4 + d] = A[tau, d]; rest stays zero.
                eng_w = nc.sync if b == 0 else nc.scalar
                eng_w.dma_start(
                    out=strided_ap(sg[b], 4, [[257, TL], [1, D]]), in_=att[:]
                )
                a0 = wp.tile([128, 64], bf16, tag=f"a0_{b}")
                nc.sync.dma_start_transpose(
                    out=a0[:], in_=strided_ap(sg[b], 0, [[256, 64], [1, 128]])
                )
                a1 = wp.tile([128, 64], bf16, tag=f"a1_{b}")
                nc.scalar.dma_start_transpose(
                    out=a1[:], in_=strided_ap(sg[b], 128, [[256, 64], [1, 128]])
                )
                abt0.append(a0)
                abt1.append(a1)

            # ---- stage 4: delay-weighted sum as banded matmul ----
            yout = wp.tile([TL, B, C, F], f32, tag="yout")
            for b in range(B):
                for c in range(C):
                    py = yp.tile([TL, F], f32, tag="py")
                    nc.tensor.matmul(
                        out=py[:], lhsT=abt0[b][:, 0:TL], rhs=xrn0[:, b, c, :],
                        start=True, stop=False,
                    )
                    nc.tensor.matmul(
                        out=py[:], lhsT=abt1[b][0 : TH - 128, 0:TL],
                        rhs=xrn1[:, b, c, :],
                        start=False, stop=True,
                    )
                    nc.vector.tensor_copy(out=yout[:, b, c, :], in_=py[:])

            nc.gpsimd.dma_start(
                out=out.ap().transpose([2, 0, 1, 3]), in_=yout[:]
            )

    nc.compile()
    return nc


def _prepare_inputs(x_mic, x_ref, w_mic, b_mic, w_ref, b_ref, w_conv):
    """Host-side sharding + weight folding. Returns in_maps (one dict/core)."""
    from ml_dtypes import bfloat16

    # padded arrays: xm rows [t0-4, t0+63), xr cols [t0-103, t0+63)
    xm_pad = np.zeros((B, C, 4 + TP, F), np.float32)
    xm_pad[:, :, 4 : 4 + T] = x_mic
    xr_pad = np.zeros((B, C, D + 3 + TP, F), np.float32)
    xr_pad[:, :, D + 3 : D + 3 + T] = x_ref

    # folded conv weights: Wc[cm, cr, i, j] = sum_h w_conv * wm~ * wr~
    wt = np.asarray(w_conv, np.float64)[0]          # (H, 5, 3)
    wtm = np.concatenate([w_mic, b_mic[:, None]], 1).astype(np.float64)  # (H,3)
    wtr = np.concatenate([w_ref, b_ref[:, None]], 1).astype(np.float64)  # (H,3)
    Wc = np.einsum("hij,hm,hr->mrij", wt, wtm, wtr)  # (3,3,5,3)

    # banded conv matrices bcv[r, q, j, tau] = Wc[q, r-tau, j]
    bcv = np.zeros((R, 3, 3, 3, TL), np.float32)
    for i in range(5):
        for j in range(3):
            bcv[np.arange(TL) + i, :, :, j, np.arange(TL)] = np.float32(
                Wc[:, :, i, j]
            )[None]
    bcv = bcv.reshape(R, NQ, 3, TL).astype(bfloat16)

    in_maps = []
    for i in range(NCORES):
        t0 = i * TL
        xm_s = xm_pad[:, :, t0 : t0 + R]          # (B,C,R,F) rows t0-4..t0+62
        xr_s = xr_pad[:, :, t0 : t0 + TH]         # (B,C,TH,F) cols t0-103..t0+62
        u = (np.arange(R) + t0 - 4 >= 0).astype(np.float32)
        v = (np.arange(TH) + t0 - D - 3 >= 0).astype(np.float32)

        xmt = np.empty((B, 3, R, F), np.float32)
        xmt[:, :C] = xm_s
        xmt[:, C] = u[:, None]
        xmt = np.ascontiguousarray(xmt.transpose(3, 0, 1, 2)).astype(bfloat16)

        xrt = np.empty((B, 3, TH, F), np.float32)
        xrt[:, :C] = xr_s
        xrt[:, C] = v[:, None]
        xrt = np.ascontiguousarray(xrt.transpose(3, 0, 1, 2)).astype(bfloat16)

        xrn = np.ascontiguousarray(xr_s.transpose(2, 0, 1, 3)).astype(bfloat16)

        in_maps.append({"xmt": xmt, "xrt": xrt, "xrn": xrn, "bcv": bcv})
    return in_maps


def kernel(**inputs):
    x_mic = np.asarray(inputs["x_mic"], np.float32)
    x_ref = np.asarray(inputs["x_ref"], np.float32)
    w_mic = np.asarray(inputs["w_mic"], np.float32)
    b_mic = np.asarray(inputs["b_mic"], np.float32)
    w_ref = np.asarray(inputs["w_ref"], np.float32)
    b_ref = np.asarray(inputs["b_ref"], np.float32)
    w_conv = np.asarray(inputs["w_conv"], np.float32)
    b_conv = np.asarray(inputs["b_conv"], np.float32)
    delay = int(np.asarray(inputs["delay"]))

    if (
        x_mic.shape != (B, C, T, F)
        or x_ref.shape != (B, C, T, F)
        or delay != D
        or w_conv.shape != (1, H, 5, 3)
    ):
        return _np_reference(
            x_mic, x_ref, w_mic, b_mic, w_ref, b_ref, w_conv, b_conv, delay
        )

    from concourse.bass_utils import run_bass_kernel_spmd

    if "nc" not in _CACHE:
        _CACHE["nc"] = _build_graph()
    nc = _CACHE["nc"]

    in_maps = _prepare_inputs(x_mic, x_ref, w_mic, b_mic, w_ref, b_ref, w_conv)
    res = run_bass_kernel_spmd(nc, in_maps, core_ids=list(range(NCORES)))

    y = np.zeros((B, C, TP, F), np.float32)
    for i in range(NCORES):
        y[:, :, i * TL : (i + 1) * TL] = res.results[i]["out"]
    return np.ascontiguousarray(y[:, :, :T]).astype(np.float32)


if __name__ == "__main__":
    rng = np.random.default_rng(0)
    ins = {
        "x_mic": rng.standard_normal((B, C, T, F), np.float32),
        "x_ref": rng.standard_normal((B, C, T, F), np.float32),
        "w_mic": rng.standard_normal((H, C), np.float32) * 0.5,
        "b_mic": rng.standard_normal((H,), np.float32) * 0.1,
        "w_ref": rng.standard_normal((H, C), np.float32) * 0.5,
        "b_ref": rng.standard_normal((H,), np.float32) * 0.1,
        "w_conv": rng.standard_normal((1, H, 5, 3), np.float32) * 0.05,
        "b_conv": rng.standard_normal((1,), np.float32) * 0.1,
        "delay": D,
    }
    got = kernel(**ins)
    want = _np_reference(**ins)
    err = np.linalg.norm(got - want) / np.linalg.norm(want)
    print("rel err vs numpy ref:", err)
